# revision 18
# baseline (speedup 1.0000x reference)
"""Causal self-attention (B=4, T=2048, C=1024, H=16) on 8 TRN2 NeuronCores.

Sharding: core = b*2 + hg  (b in 0..3 batches, hg in 0..1 head-groups of 8
heads).  Each core computes QKV projection, flash-style causal attention and
the c_proj partial product for its 8 heads of one batch; the host sums the
two partial c_proj outputs per batch (tensor-parallel reduction) while
gathering.

Device layouts (SBUF partition dim first):
  xT   [C, T]   x transposed (host-prepped), bf16
  Q^T/K^T [512, T] channel-major via matmul(lhsT=w, rhs=xT)
  V    [T, 512] token-major via matmul(lhsT=xT_tile, rhs=wv), augmented with
       a ones column per head -> PV matmul yields both O^T and the softmax
       row-sum Z in one PSUM tile [65, 512].
  S^T  [j, i] = matmul(lhsT=K^T[d, j], rhs=Q^T[d, i]), K=64 contraction; the
       even/odd heads of a pair sit at partitions 0-63 / 64-127 so the two
       K=64 matmuls run concurrently in separate PE row groups.
"""

import os

import numpy as np
import ml_dtypes

import concourse.bass as bass
from concourse import bacc
import concourse.mybir as mybir
import concourse.tile as tile
from concourse.bass_utils import run_bass_kernel_spmd

B, T, C = 4, 2048, 1024
H, D = 16, 64
HG = 2                    # head-groups (tensor parallel)
HL = H // HG              # heads per core
CL = HL * D               # 512 local channels per section
N_CORES = 8
KT_C = C // 128           # 8 contraction tiles over C
MT_QK = 2 * CL // 128     # 8 output tiles for [Q|K] channels
TT128 = T // 128          # 16
TT512 = T // 512          # 4
SCALE = 1.0 / 8.0         # 1/sqrt(D)

BF16 = mybir.dt.bfloat16
F32 = mybir.dt.float32
EXP = mybir.ActivationFunctionType.Exp

LAST_EXEC_NS = None
_CACHE = {}


def _ensure_ntff_hook():
    """The agent image's ``antenv`` package lacks ``axon_hooks``, so the
    boot-time NTFF-profile-hook registration silently degraded.  Inject an
    in-process module and register the ctypes hook so trace=True works."""
    import sys
    import types

    try:
        from antenv import axon_hooks  # noqa: F401
        return
    except ImportError:
        pass
    mod = types.ModuleType("antenv.axon_hooks")
    mod._hook = None

    def set_axon_ntff_profile_hook(h):
        mod._hook = h

    def get_axon_ntff_profile_hook():
        return mod._hook

    mod.set_axon_ntff_profile_hook = set_axon_ntff_profile_hook
    mod.get_axon_ntff_profile_hook = get_axon_ntff_profile_hook
    sys.modules["antenv.axon_hooks"] = mod
    try:
        from trn_agent_boot.trn_boot import _ntff_profile_via_ctypes

        hook = _ntff_profile_via_ctypes("/opt/axon/libaxon_pjrt.so")
        if hook is not None:
            set_axon_ntff_profile_hook(hook)
    except Exception:
        pass


def _build():
    nc = bacc.Bacc()
    xT = nc.declare_dram_parameter("xT", [C, T], BF16, isOutput=False)
    wqk = nc.declare_dram_parameter("wqk", [C, 2 * CL], BF16, isOutput=False)
    wv = nc.declare_dram_parameter("wv", [C, CL], BF16, isOutput=False)
    bqk = nc.declare_dram_parameter("bqk", [128, MT_QK], F32, isOutput=False)
    wp = nc.declare_dram_parameter("wp", [CL, C], BF16, isOutput=False)
    tri = nc.declare_dram_parameter("tri", [128, 128], BF16, isOutput=False)
    out = nc.declare_dram_parameter("out", [T, C], F32, isOutput=True)

    with tile.TileContext(nc) as tc:
        with (
            tc.tile_pool(name="big", bufs=1) as big,
            tc.tile_pool(name="work", bufs=6) as work,
            tc.tile_pool(name="small", bufs=4) as small,
            tc.tile_pool(name="outp", bufs=3) as outp,
            tc.tile_pool(name="ps", bufs=8, space="PSUM") as ps,
        ):
            xT_sb = big.tile([128, KT_C, T], BF16)
            wqk_sb = big.tile([128, KT_C, 2 * CL], BF16)
            wv_sb = big.tile([128, KT_C, CL], BF16)
            wp_sb = big.tile([128, CL // 128, C], BF16)
            qt_sb = big.tile([128, HL // 2, T], BF16)
            kt_sb = big.tile([128, HL // 2, T], BF16)
            vaug_sb = big.tile([128, TT128, HL, D + 1], BF16)
            at_sb = big.tile([128, HL // 2, T], BF16)
            bqk_sb = big.tile([128, MT_QK], F32)
            tri_sb = big.tile([128, 128], BF16)
            ones_sb = big.tile([128, 512], BF16)

            nc.vector.memset(ones_sb, 1.0)
            nc.vector.memset(vaug_sb[:, :, :, D], 1.0)
            for k in range(KT_C):
                for q in range(4):
                    nc.sync.dma_start(
                        out=xT_sb[:, k, q * 512:(q + 1) * 512],
                        in_=xT[k * 128:(k + 1) * 128, q * 512:(q + 1) * 512],
                    )
                nc.sync.dma_start(out=wqk_sb[:, k, :], in_=wqk[k * 128:(k + 1) * 128, :])
                nc.sync.dma_start(out=wv_sb[:, k, :], in_=wv[k * 128:(k + 1) * 128, :])
            for k in range(CL // 128):
                nc.sync.dma_start(out=wp_sb[:, k, :], in_=wp[k * 128:(k + 1) * 128, :])
            nc.sync.dma_start(out=bqk_sb, in_=bqk[:, :])
            nc.sync.dma_start(out=tri_sb, in_=tri[:, :])

            def qk_mtile(m):
                # Q^T / K^T channels [m*128, (m+1)*128) over all tokens.
                dst = qt_sb if m < MT_QK // 2 else kt_sb
                mi = m % (MT_QK // 2)
                for t5 in range(TT512):
                    acc = ps.tile([128, 512], F32, tag="pq", bufs=2,
                                  name=f"qk_{m}_{t5}")
                    for k in range(KT_C):
                        nc.tensor.matmul(
                            acc,
                            wqk_sb[:, k, m * 128:(m + 1) * 128],
                            xT_sb[:, k, t5 * 512:(t5 + 1) * 512],
                            start=(k == 0),
                            stop=(k == KT_C - 1),
                        )
                    nc.vector.tensor_scalar_add(
                        dst[:, mi, t5 * 512:(t5 + 1) * 512], acc, bqk_sb[:, m:m + 1]
                    )

            def v_ttile(tt):
                acc = ps.tile([128, CL], F32, tag="pq", bufs=2, name=f"v_{tt}")
                for k in range(KT_C):
                    nc.tensor.matmul(
                        acc,
                        xT_sb[:, k, tt * 128:(tt + 1) * 128],
                        wv_sb[:, k, :],
                        start=(k == 0),
                        stop=(k == KT_C - 1),
                    )
                nc.vector.tensor_copy(
                    vaug_sb[:, tt, :, 0:D],
                    acc.rearrange("p (h d) -> p h d", d=D),
                )

            def attention(it, hp):
                i0 = it * 512
                n_j = 4 * (it + 1)
                o_e = ps.tile([D + 1, 512], F32, tag="o", bufs=2,
                              name=f"oe_{it}_{hp}")
                o_o = ps.tile([D + 1, 512], F32, tag="o", bufs=2,
                              name=f"oo_{it}_{hp}")
                for jt in range(n_j):
                    j0 = jt * 128
                    off = max(0, j0 - i0)
                    s_p = ps.tile([128, 2, 512], F32, tag="s", bufs=2,
                                  name=f"s_{it}_{hp}_{jt}")
                    nc.tensor.matmul(
                        s_p[:, 0, off:],
                        kt_sb[0:64, hp, j0:j0 + 128],
                        qt_sb[0:64, hp, i0 + off:i0 + 512],
                    )
                    nc.tensor.matmul(
                        s_p[:, 1, off:],
                        kt_sb[64:128, hp, j0:j0 + 128],
                        qt_sb[64:128, hp, i0 + off:i0 + 512],
                    )
                    p_p = work.tile([128, 2, 512], BF16, tag="pt",
                                    name=f"p_{it}_{hp}_{jt}")
                    nc.scalar.activation(
                        p_p[:, :, off:], s_p[:, :, off:], EXP, scale=SCALE
                    )
                    if j0 >= i0:
                        nc.vector.tensor_mul(
                            p_p[:, 0, off:off + 128], p_p[:, 0, off:off + 128], tri_sb
                        )
                        nc.vector.tensor_mul(
                            p_p[:, 1, off:off + 128], p_p[:, 1, off:off + 128], tri_sb
                        )
                    nc.tensor.matmul(
                        o_e[:, off:],
                        vaug_sb[:, jt, 2 * hp, :],
                        p_p[:, 0, off:],
                        start=(jt == 0),
                        stop=(jt == n_j - 1),
                    )
                    nc.tensor.matmul(
                        o_o[:, off:],
                        vaug_sb[:, jt, 2 * hp + 1, :],
                        p_p[:, 1, off:],
                        start=(jt == 0),
                        stop=(jt == n_j - 1),
                    )
                # Normalize: broadcast Z across 64 partitions with a K=1
                # matmul on a bf16 copy, then 1/Z via the fast Newton
                # reciprocal as the PSUM->SBUF evacuation step.
                z_e = small.tile([D + 1, 512], BF16, tag="r", name=f"ze_{it}_{hp}")
                z_o = small.tile([D + 1, 512], BF16, tag="r", name=f"zo_{it}_{hp}")
                nc.vector.tensor_copy(z_e[D:D + 1, :], o_e[D:D + 1, :])
                nc.vector.tensor_copy(z_o[D:D + 1, :], o_o[D:D + 1, :])
                zb_e = ps.tile([D, 512], F32, tag="pq", bufs=2,
                               name=f"zbe_{it}_{hp}")
                zb_o = ps.tile([D, 512], F32, tag="pq", bufs=2,
                               name=f"zbo_{it}_{hp}")
                nc.tensor.matmul(zb_e, ones_sb[64:65, 0:D], z_e[D:D + 1, :])
                nc.tensor.matmul(zb_o, ones_sb[64:65, 0:D], z_o[D:D + 1, :])
                rs_e = small.tile([D, 512], F32, tag="rs", name=f"rse_{it}_{hp}")
                rs_o = small.tile([D, 512], F32, tag="rs", name=f"rso_{it}_{hp}")
                nc.vector.reciprocal_approx_fast(rs_e, zb_e)
                nc.vector.reciprocal_approx_fast(rs_o, zb_o)
                nc.vector.tensor_mul(
                    at_sb[0:64, hp, i0:i0 + 512], o_e[0:D, :], rs_e
                )
                tmp = small.tile([D, 512], BF16, tag="tmp", name=f"tmp_{it}_{hp}")
                nc.vector.tensor_mul(tmp, o_o[0:D, :], rs_o)
                nc.sync.dma_start(out=at_sb[64:128, hp, i0:i0 + 512], in_=tmp)

            def proj(tt):
                for ch in range(2):
                    acc = ps.tile([128, 512], F32, tag="pq", bufs=2,
                                  name=f"pr_{tt}_{ch}")
                    for kt in range(CL // 128):
                        nc.tensor.matmul(
                            acc,
                            at_sb[:, kt, tt * 128:(tt + 1) * 128],
                            wp_sb[:, kt, ch * 512:(ch + 1) * 512],
                            start=(kt == 0),
                            stop=(kt == CL // 128 - 1),
                        )
                    ob = outp.tile([128, 512], F32, tag="ob", name=f"ob_{tt}_{ch}")
                    nc.vector.tensor_copy(ob, acc)
                    nc.sync.dma_start(
                        out=out[tt * 128:(tt + 1) * 128, ch * 512:(ch + 1) * 512],
                        in_=ob,
                    )

            # Emission order interleaves QKV with attention so ACT exp work
            # starts while the PE is still on projection matmuls.
            qk_mtile(0)
            qk_mtile(4)
            for tt in range(TT128):
                v_ttile(tt)
            for m in (1, 5, 2, 6, 3, 7):
                qk_mtile(m)
            for it in range(TT512):
                for hp in range(HL // 2):
                    attention(it, hp)
                for tt in range(it * 4, it * 4 + 4):
                    proj(tt)

    nc.compile()
    return nc


def _get_nc():
    if "nc" not in _CACHE:
        _CACHE["nc"] = _build()
    return _CACHE["nc"]


def make_in_maps(x, w_attn, b_attn, w_proj, b_proj):
    bf = ml_dtypes.bfloat16
    x = np.asarray(x, np.float32)
    w_attn = np.asarray(w_attn, np.float32)
    b_attn = np.asarray(b_attn, np.float32)
    w_proj = np.asarray(w_proj, np.float32)
    b_proj = np.asarray(b_proj, np.float32)
    tri = np.triu(np.ones((128, 128), np.float32)).astype(bf)
    in_maps = []
    for core in range(N_CORES):
        b, hg = divmod(core, 2)
        hs = hg * CL
        xT = np.ascontiguousarray(x[b].T).astype(bf)
        wqk = np.concatenate(
            [w_attn[:, hs:hs + CL], w_attn[:, C + hs:C + hs + CL]], axis=1
        ).astype(bf)
        wv = np.ascontiguousarray(w_attn[:, 2 * C + hs:2 * C + hs + CL]).astype(bf)
        bqk = (
            np.concatenate([b_attn[hs:hs + CL], b_attn[C + hs:C + hs + CL]])
            .reshape(MT_QK, 128)
            .T.astype(np.float32)
            .copy()
        )
        wp = np.ascontiguousarray(w_proj[hs:hs + CL, :]).astype(bf)
        in_maps.append(dict(xT=xT, wqk=wqk, wv=wv, bqk=bqk, wp=wp, tri=tri))
    return in_maps


def output_bias(w_attn, b_attn, w_proj, b_proj):
    """V-bias commutes through softmax (rows sum to 1), so it and the proj
    bias fold into one output-bias vector added after the gather."""
    bv = b_attn[2 * C:3 * C].astype(np.float64)
    return (bv @ w_proj.astype(np.float64) + b_proj.astype(np.float64)).astype(
        np.float32
    )


def kernel(x, w_attn, b_attn, w_proj, b_proj):
    global LAST_EXEC_NS
    nc = _get_nc()
    in_maps = make_in_maps(x, w_attn, b_attn, w_proj, b_proj)
    trace = bool(int(os.environ.get("BASS_KERNEL_TRACE", "0")))
    if trace:
        _ensure_ntff_hook()
    res = run_bass_kernel_spmd(nc, in_maps, list(range(N_CORES)), trace=trace)
    LAST_EXEC_NS = res.exec_time_ns
    outs = [r["out"].astype(np.float32) for r in res.results]
    bias = output_bias(
        np.asarray(w_attn, np.float32), np.asarray(b_attn, np.float32),
        np.asarray(w_proj, np.float32), np.asarray(b_proj, np.float32),
    )
    return np.stack([outs[2 * b] + outs[2 * b + 1] + bias for b in range(B)])


# revision 19
# speedup vs baseline: 1.0602x; 1.0602x over previous
"""Causal self-attention (B=4, T=2048, C=1024, H=16) on 8 TRN2 NeuronCores.

Sharding: core = b*2 + hg  (b in 0..3 batches, hg in 0..1 head-groups of 8
heads).  Each core computes QKV projection, flash-style causal attention and
the c_proj partial product for its 8 heads of one batch; the host sums the
two partial c_proj outputs per batch (tensor-parallel reduction) while
gathering.

Device layouts (SBUF partition dim first):
  xT   [C, T]   x transposed (host-prepped), bf16
  Q^T/K^T [512, T] channel-major via matmul(lhsT=w, rhs=xT)
  V    [T, 512] token-major via matmul(lhsT=xT_tile, rhs=wv), augmented with
       a ones column per head -> PV matmul yields both O^T and the softmax
       row-sum Z in one PSUM tile [65, 512].
  S^T  [j, i] = matmul(lhsT=K^T[d, j], rhs=Q^T[d, i]), K=64 contraction; the
       even/odd heads of a pair sit at partitions 0-63 / 64-127 so the two
       K=64 matmuls run concurrently in separate PE row groups.
"""

import os

import numpy as np
import ml_dtypes

import concourse.bass as bass
from concourse import bacc
import concourse.mybir as mybir
import concourse.tile as tile
from concourse.bass_utils import run_bass_kernel_spmd

B, T, C = 4, 2048, 1024
H, D = 16, 64
HG = 2                    # head-groups (tensor parallel)
HL = H // HG              # heads per core
CL = HL * D               # 512 local channels per section
N_CORES = 8
KT_C = C // 128           # 8 contraction tiles over C
MT_QK = 2 * CL // 128     # 8 output tiles for [Q|K] channels
TT128 = T // 128          # 16
TT512 = T // 512          # 4
SCALE = 1.0 / 8.0         # 1/sqrt(D)

BF16 = mybir.dt.bfloat16
F32 = mybir.dt.float32
EXP = mybir.ActivationFunctionType.Exp

LAST_EXEC_NS = None
_CACHE = {}


def _ensure_ntff_hook():
    """The agent image's ``antenv`` package lacks ``axon_hooks``, so the
    boot-time NTFF-profile-hook registration silently degraded.  Inject an
    in-process module and register the ctypes hook so trace=True works."""
    import sys
    import types

    try:
        from antenv import axon_hooks  # noqa: F401
        return
    except ImportError:
        pass
    mod = types.ModuleType("antenv.axon_hooks")
    mod._hook = None

    def set_axon_ntff_profile_hook(h):
        mod._hook = h

    def get_axon_ntff_profile_hook():
        return mod._hook

    mod.set_axon_ntff_profile_hook = set_axon_ntff_profile_hook
    mod.get_axon_ntff_profile_hook = get_axon_ntff_profile_hook
    sys.modules["antenv.axon_hooks"] = mod
    try:
        from trn_agent_boot.trn_boot import _ntff_profile_via_ctypes

        hook = _ntff_profile_via_ctypes("/opt/axon/libaxon_pjrt.so")
        if hook is not None:
            set_axon_ntff_profile_hook(hook)
    except Exception:
        pass


def _build():
    nc = bacc.Bacc()
    xT = nc.declare_dram_parameter("xT", [C, T], BF16, isOutput=False)
    wqk = nc.declare_dram_parameter("wqk", [C, 2 * CL], BF16, isOutput=False)
    wv = nc.declare_dram_parameter("wv", [C, CL], BF16, isOutput=False)
    bqk = nc.declare_dram_parameter("bqk", [128, MT_QK], F32, isOutput=False)
    wp = nc.declare_dram_parameter("wp", [CL, C], BF16, isOutput=False)
    tri = nc.declare_dram_parameter("tri", [128, 128], BF16, isOutput=False)
    out = nc.declare_dram_parameter("out", [T, C], F32, isOutput=True)

    with tile.TileContext(nc) as tc:
        with (
            tc.tile_pool(name="big", bufs=1) as big,
            tc.tile_pool(name="work", bufs=6) as work,
            tc.tile_pool(name="small", bufs=4) as small,
            tc.tile_pool(name="outp", bufs=3) as outp,
            tc.tile_pool(name="ps", bufs=8, space="PSUM") as ps,
        ):
            xT_sb = big.tile([128, KT_C, T], BF16)
            wqk_sb = big.tile([128, KT_C, 2 * CL], BF16)
            wv_sb = big.tile([128, KT_C, CL], BF16)
            wp_sb = big.tile([128, CL // 128, C], BF16)
            qt_sb = big.tile([128, HL // 2, T], BF16)
            kt_sb = big.tile([128, HL // 2, T], BF16)
            vaug_sb = big.tile([128, TT128, HL, D + 1], BF16)
            at_sb = big.tile([128, HL // 2, T], BF16)
            bqk_sb = big.tile([128, MT_QK], F32)
            tri_sb = big.tile([128, 128], BF16)
            ones_sb = big.tile([128, 512], BF16)

            nc.vector.memset(ones_sb, 1.0)
            nc.vector.memset(vaug_sb[:, :, :, D], 1.0)
            for k in range(KT_C):
                nc.sync.dma_start(out=xT_sb[:, k, :], in_=xT[k * 128:(k + 1) * 128, :])
                nc.sync.dma_start(out=wqk_sb[:, k, :], in_=wqk[k * 128:(k + 1) * 128, :])
                nc.sync.dma_start(out=wv_sb[:, k, :], in_=wv[k * 128:(k + 1) * 128, :])
            for k in range(CL // 128):
                nc.sync.dma_start(out=wp_sb[:, k, :], in_=wp[k * 128:(k + 1) * 128, :])
            nc.sync.dma_start(out=bqk_sb, in_=bqk[:, :])
            nc.sync.dma_start(out=tri_sb, in_=tri[:, :])

            def qk_mtile(m):
                # Q^T / K^T channels [m*128, (m+1)*128) over all tokens.
                dst = qt_sb if m < MT_QK // 2 else kt_sb
                mi = m % (MT_QK // 2)
                for t5 in range(TT512):
                    acc = ps.tile([128, 512], F32, tag="pq", bufs=2,
                                  name=f"qk_{m}_{t5}")
                    for k in range(KT_C):
                        nc.tensor.matmul(
                            acc,
                            wqk_sb[:, k, m * 128:(m + 1) * 128],
                            xT_sb[:, k, t5 * 512:(t5 + 1) * 512],
                            start=(k == 0),
                            stop=(k == KT_C - 1),
                        )
                    nc.vector.tensor_scalar_add(
                        dst[:, mi, t5 * 512:(t5 + 1) * 512], acc, bqk_sb[:, m:m + 1]
                    )

            def v_ttile(tt):
                acc = ps.tile([128, CL], F32, tag="pq", bufs=2, name=f"v_{tt}")
                for k in range(KT_C):
                    nc.tensor.matmul(
                        acc,
                        xT_sb[:, k, tt * 128:(tt + 1) * 128],
                        wv_sb[:, k, :],
                        start=(k == 0),
                        stop=(k == KT_C - 1),
                    )
                nc.vector.tensor_copy(
                    vaug_sb[:, tt, :, 0:D],
                    acc.rearrange("p (h d) -> p h d", d=D),
                )

            def attention(it, hp):
                i0 = it * 512
                n_j = 4 * (it + 1)
                o_e = ps.tile([D + 1, 512], F32, tag="o", bufs=2,
                              name=f"oe_{it}_{hp}")
                o_o = ps.tile([D + 1, 512], F32, tag="o", bufs=2,
                              name=f"oo_{it}_{hp}")
                for jt in range(n_j):
                    j0 = jt * 128
                    off = max(0, j0 - i0)
                    s_p = ps.tile([128, 2, 512], F32, tag="s", bufs=2,
                                  name=f"s_{it}_{hp}_{jt}")
                    nc.tensor.matmul(
                        s_p[:, 0, off:],
                        kt_sb[0:64, hp, j0:j0 + 128],
                        qt_sb[0:64, hp, i0 + off:i0 + 512],
                    )
                    nc.tensor.matmul(
                        s_p[:, 1, off:],
                        kt_sb[64:128, hp, j0:j0 + 128],
                        qt_sb[64:128, hp, i0 + off:i0 + 512],
                    )
                    p_p = work.tile([128, 2, 512], BF16, tag="pt",
                                    name=f"p_{it}_{hp}_{jt}")
                    nc.scalar.activation(
                        p_p[:, :, off:], s_p[:, :, off:], EXP, scale=SCALE
                    )
                    if j0 >= i0:
                        nc.vector.tensor_mul(
                            p_p[:, 0, off:off + 128], p_p[:, 0, off:off + 128], tri_sb
                        )
                        nc.vector.tensor_mul(
                            p_p[:, 1, off:off + 128], p_p[:, 1, off:off + 128], tri_sb
                        )
                    nc.tensor.matmul(
                        o_e[:, off:],
                        vaug_sb[:, jt, 2 * hp, :],
                        p_p[:, 0, off:],
                        start=(jt == 0),
                        stop=(jt == n_j - 1),
                    )
                    nc.tensor.matmul(
                        o_o[:, off:],
                        vaug_sb[:, jt, 2 * hp + 1, :],
                        p_p[:, 1, off:],
                        start=(jt == 0),
                        stop=(jt == n_j - 1),
                    )
                # Normalize: broadcast Z across 64 partitions with a K=1
                # matmul on a bf16 copy, then 1/Z via the fast Newton
                # reciprocal as the PSUM->SBUF evacuation step.
                z_e = small.tile([D + 1, 512], BF16, tag="r", name=f"ze_{it}_{hp}")
                z_o = small.tile([D + 1, 512], BF16, tag="r", name=f"zo_{it}_{hp}")
                nc.vector.tensor_copy(z_e[D:D + 1, :], o_e[D:D + 1, :])
                nc.vector.tensor_copy(z_o[D:D + 1, :], o_o[D:D + 1, :])
                zb_e = ps.tile([D, 512], F32, tag="pq", bufs=2,
                               name=f"zbe_{it}_{hp}")
                zb_o = ps.tile([D, 512], F32, tag="pq", bufs=2,
                               name=f"zbo_{it}_{hp}")
                nc.tensor.matmul(zb_e, ones_sb[64:65, 0:D], z_e[D:D + 1, :])
                nc.tensor.matmul(zb_o, ones_sb[64:65, 0:D], z_o[D:D + 1, :])
                rs_e = small.tile([D, 512], F32, tag="rs", name=f"rse_{it}_{hp}")
                rs_o = small.tile([D, 512], F32, tag="rs", name=f"rso_{it}_{hp}")
                nc.vector.reciprocal_approx_fast(rs_e, zb_e)
                nc.vector.reciprocal_approx_fast(rs_o, zb_o)
                nc.vector.tensor_mul(
                    at_sb[0:64, hp, i0:i0 + 512], o_e[0:D, :], rs_e
                )
                tmp = small.tile([D, 512], BF16, tag="tmp", name=f"tmp_{it}_{hp}")
                nc.vector.tensor_mul(tmp, o_o[0:D, :], rs_o)
                nc.sync.dma_start(out=at_sb[64:128, hp, i0:i0 + 512], in_=tmp)

            def proj(tt):
                for ch in range(2):
                    acc = ps.tile([128, 512], F32, tag="pq", bufs=2,
                                  name=f"pr_{tt}_{ch}")
                    for kt in range(CL // 128):
                        nc.tensor.matmul(
                            acc,
                            at_sb[:, kt, tt * 128:(tt + 1) * 128],
                            wp_sb[:, kt, ch * 512:(ch + 1) * 512],
                            start=(kt == 0),
                            stop=(kt == CL // 128 - 1),
                        )
                    ob = outp.tile([128, 512], F32, tag="ob", name=f"ob_{tt}_{ch}")
                    nc.vector.tensor_copy(ob, acc)
                    nc.sync.dma_start(
                        out=out[tt * 128:(tt + 1) * 128, ch * 512:(ch + 1) * 512],
                        in_=ob,
                    )

            # Emission order interleaves QKV with attention so ACT exp work
            # starts while the PE is still on projection matmuls.
            qk_mtile(0)
            qk_mtile(4)
            for tt in range(TT128):
                v_ttile(tt)
            for m in (1, 5, 2, 6, 3, 7):
                qk_mtile(m)
            for it in range(TT512):
                for hp in range(HL // 2):
                    attention(it, hp)
                for tt in range(it * 4, it * 4 + 4):
                    proj(tt)

    nc.compile()
    return nc


def _get_nc():
    if "nc" not in _CACHE:
        _CACHE["nc"] = _build()
    return _CACHE["nc"]


def make_in_maps(x, w_attn, b_attn, w_proj, b_proj):
    bf = ml_dtypes.bfloat16
    x = np.asarray(x, np.float32)
    w_attn = np.asarray(w_attn, np.float32)
    b_attn = np.asarray(b_attn, np.float32)
    w_proj = np.asarray(w_proj, np.float32)
    b_proj = np.asarray(b_proj, np.float32)
    tri = np.triu(np.ones((128, 128), np.float32)).astype(bf)
    in_maps = []
    for core in range(N_CORES):
        b, hg = divmod(core, 2)
        hs = hg * CL
        xT = np.ascontiguousarray(x[b].T).astype(bf)
        wqk = np.concatenate(
            [w_attn[:, hs:hs + CL], w_attn[:, C + hs:C + hs + CL]], axis=1
        ).astype(bf)
        wv = np.ascontiguousarray(w_attn[:, 2 * C + hs:2 * C + hs + CL]).astype(bf)
        bqk = (
            np.concatenate([b_attn[hs:hs + CL], b_attn[C + hs:C + hs + CL]])
            .reshape(MT_QK, 128)
            .T.astype(np.float32)
            .copy()
        )
        wp = np.ascontiguousarray(w_proj[hs:hs + CL, :]).astype(bf)
        in_maps.append(dict(xT=xT, wqk=wqk, wv=wv, bqk=bqk, wp=wp, tri=tri))
    return in_maps


def output_bias(w_attn, b_attn, w_proj, b_proj):
    """V-bias commutes through softmax (rows sum to 1), so it and the proj
    bias fold into one output-bias vector added after the gather."""
    bv = b_attn[2 * C:3 * C].astype(np.float64)
    return (bv @ w_proj.astype(np.float64) + b_proj.astype(np.float64)).astype(
        np.float32
    )


def kernel(x, w_attn, b_attn, w_proj, b_proj):
    global LAST_EXEC_NS
    nc = _get_nc()
    in_maps = make_in_maps(x, w_attn, b_attn, w_proj, b_proj)
    trace = bool(int(os.environ.get("BASS_KERNEL_TRACE", "0")))
    if trace:
        _ensure_ntff_hook()
    res = run_bass_kernel_spmd(nc, in_maps, list(range(N_CORES)), trace=trace)
    LAST_EXEC_NS = res.exec_time_ns
    outs = [r["out"].astype(np.float32) for r in res.results]
    bias = output_bias(
        np.asarray(w_attn, np.float32), np.asarray(b_attn, np.float32),
        np.asarray(w_proj, np.float32), np.asarray(b_proj, np.float32),
    )
    return np.stack([outs[2 * b] + outs[2 * b + 1] + bias for b in range(B)])
